# revision 1
# baseline (speedup 1.0000x reference)
"""Trainium2 Bass kernel for nn_Alembic_Layer_30923764531483 (dense_cnn).

Reference computes, per batch b (512) and filter f (3): windowed-sinc bandpass
taps (K=101) from 2 scalars, then a depthwise 'same' correlation over 32
channels of length-500 signals.  out[b,f,c,:] = corr(x[b, (32f+c)//3, :],
taps[b,f,:]).

Strategy (8 cores, data parallel over b, 64 b/core):
  - Host (numpy, free): compute taps exactly as the reference; build
    (a) 18 overlapping time-major x windows per b: XW[b,w] = xpad.T rows
        [s_w, s_w+128) x 32 channels, s_w in {0,28,...,448,472}  (bf16)
    (b) a window-invariant Toeplitz tile per b: T3[b][j, f*28+l] =
        taps[b,f,j-l] (valid 0<=j-l<=100)                        (bf16)
  - Device: each window is ONE 128-contraction matmul
        psum[c, (f,l)] = sum_j XW[j,c] * T3[j, (f,l)]
    col-tiled 4 batches across the PE array (tile_position=(0,32i)) so the
    full 128-partition PSUM is used; DVE/ACT evacuate into (c, f, 500)
    layout; contiguous DMA out.
  - Host: gather output channels per the grouped-conv routing, concat cores.
"""

import sys

sys.path.insert(0, "/opt/trn_rl_repo")

import numpy as np
import ml_dtypes

B, C, L, FS, K, F = 512, 32, 500, 128, 101, 3
NCORES = 8
BLOC = B // NCORES          # 64 batches per core
W = 28                      # outputs per window
NWIN = 18                   # windows per batch
WIN_STARTS = [28 * w for w in range(17)] + [472]
NG = BLOC // 4              # 16 groups of 4 batches
PI = np.float32(np.pi)

_CACHE = {}


def _make_taps_np(fp):
    """Mirror reference._make_taps in numpy float32. fp: (B, 3, 2)."""
    lows = fp[:, :, 0].astype(np.float32) / np.float32(0.5 * FS)
    highs = fp[:, :, 1].astype(np.float32) / np.float32(0.5 * FS)
    n = np.arange(K, dtype=np.float32) - np.float32((K - 1) / 2.0)
    c = (K - 1) // 2
    n_safe = n.copy()
    n_safe[c] = 1.0
    taps = (
        np.sin(PI * n * highs[..., None]) - np.sin(PI * n * lows[..., None])
    ) / (PI * n_safe)
    taps[:, :, c] = highs - lows
    win = 0.5 - 0.5 * np.cos(2.0 * PI * np.arange(K, dtype=np.float32) / K)
    return (taps * win).astype(np.float32)  # (B, 3, K)


def _build_program():
    import concourse.bass as bass
    import concourse.tile as tile
    from concourse import bacc, mybir

    bf16 = mybir.dt.bfloat16
    f32 = mybir.dt.float32

    nc = bacc.Bacc("TRN2", target_bir_lowering=False, debug=False)

    # xw: moving operand — 18 overlapping time-major windows per batch
    # t3: stationary Toeplitz, M = f*28 + l' (84 cols)
    # out layout: partitions (f,l') x free (w,c) in bf16; host un-permutes+upcasts.
    xw_d = nc.dram_tensor("xw", [NG, 128, 4, NWIN, C], bf16, kind="ExternalInput")
    t3_d = nc.dram_tensor("t3", [NG, 128, 4, F, W], bf16, kind="ExternalInput")
    out_d = nc.dram_tensor("out", [NG, 84, 4, NWIN * C], bf16, kind="ExternalOutput")

    with tile.TileContext(nc) as tc:
        with (
            tc.tile_pool(name="xw", bufs=4) as xw_pool,
            tc.tile_pool(name="t3", bufs=4) as t3_pool,
            tc.tile_pool(name="res", bufs=3) as res_pool,
            tc.tile_pool(name="ps", bufs=4, space=bass.MemorySpace.PSUM) as ps_pool,
        ):
            for g in range(NG):
                xw_t = xw_pool.tile([128, 4, NWIN, C], bf16)
                t3_t = t3_pool.tile([128, 4, F, W], bf16)
                nc.gpsimd.dma_start(out=t3_t[:], in_=t3_d[g])
                nc.sync.dma_start(out=xw_t[:], in_=xw_d[g])

                if True:
                    res_t = res_pool.tile([84, 4, NWIN * C], bf16)
                    for i4 in range(4):
                        i = i4
                        b = 4 * g + i
                        ps_t = ps_pool.tile([84, 2, 512], f32)
                        nc.tensor.matmul(
                            ps_t[:, 0, :],
                            lhsT=t3_t[:, i, :, :].rearrange("p f w -> p (f w)"),
                            rhs=xw_t[:, i, 0:16, :].rearrange("p w c -> p (w c)"),
                            start=True,
                            stop=True,
                        )
                        nc.tensor.matmul(
                            ps_t[:, 1, 0:64],
                            lhsT=t3_t[:, i, :, :].rearrange("p f w -> p (f w)"),
                            rhs=xw_t[:, i, 16:NWIN, :].rearrange("p w c -> p (w c)"),
                            start=True,
                            stop=True,
                        )
                        src = ps_t[:, :, :].rearrange("p a n -> p (a n)")[
                            :, 0 : NWIN * C
                        ]
                        if i4 % 2 == 0:
                            nc.vector.tensor_copy(res_t[:, i4, :], src)
                        else:
                            nc.scalar.copy(res_t[:, i4, :], src)
                    nc.scalar.dma_start(out=out_d[g], in_=res_t[:, :, :])

    nc.compile()
    return nc


def _get_program():
    if "nc" not in _CACHE:
        _CACHE["nc"] = _build_program()
    return _CACHE["nc"]


def _prep_core_inputs(x_core, taps_core):
    """x_core: (64, C, L) f32; taps_core: (64, 3, K) f32 -> input map."""
    # padded, time-major windows
    xp = np.zeros((BLOC, C, 600), dtype=np.float32)
    xp[:, :, 50:550] = x_core
    # XW[b, w, j, c] = xp[b, c, s_w + j]
    starts = np.asarray(WIN_STARTS)
    # gather windows: (BLOC, NWIN, C, 128) then -> (BLOC, NWIN, 128, C)
    idx = starts[:, None] + np.arange(128)[None, :]          # (NWIN, 128)
    xw = xp[:, :, idx]                                       # (BLOC, C, NWIN, 128)
    xw = xw.transpose(0, 2, 3, 1)                            # (BLOC, NWIN, 128, C)
    # group layout (NG, 128, 4, NWIN, C)
    xw_g = np.ascontiguousarray(
        xw.reshape(NG, 4, NWIN, 128, C).transpose(0, 3, 1, 2, 4)
    ).astype(ml_dtypes.bfloat16)

    # T3[j, b, f*28 + l] = taps[b, f, j - l] for l in [0,28), 0 <= j-l <= 100
    jj = np.arange(128)[:, None] - np.arange(W)[None, :]     # (128, W)
    valid = (jj >= 0) & (jj < K)
    t3 = taps_core[:, :, np.clip(jj, 0, K - 1)] * valid[None, None]  # (B,3,128,W)
    # (NG, 128, 4, F, W)
    t3_g = np.ascontiguousarray(
        t3.reshape(NG, 4, F, 128, W).transpose(0, 3, 1, 2, 4)
    ).astype(ml_dtypes.bfloat16)
    return {"xw": xw_g, "t3": t3_g}


def _install_ntff_hook():
    """Provide antenv.axon_hooks (missing on this image) so
    run_bass_kernel_spmd's trace=True path can capture NTFF profiles."""
    import sys as _sys

    if "antenv.axon_hooks" in _sys.modules:
        return
    import contextlib
    import ctypes
    import types

    try:
        lib = ctypes.CDLL("/opt/axon/libaxon_pjrt.so")
        if not hasattr(lib, "axon_start_nrt_profile"):
            return
    except OSError:
        return
    lib.axon_start_nrt_profile.argtypes = [
        ctypes.POINTER(ctypes.c_int64),
        ctypes.c_size_t,
    ]
    lib.axon_start_nrt_profile.restype = ctypes.c_int64
    lib.axon_stop_nrt_profile.argtypes = [ctypes.c_char_p]
    lib.axon_stop_nrt_profile.restype = ctypes.c_int64

    @contextlib.contextmanager
    def _hook(output_dir, device_ids):
        import jax

        jax.devices()
        if device_ids:
            ids = (ctypes.c_int64 * len(device_ids))(*device_ids)
            rc = lib.axon_start_nrt_profile(ids, len(device_ids))
        else:
            rc = lib.axon_start_nrt_profile(None, 0)
        if rc != 0:
            raise RuntimeError(f"axon_start_nrt_profile rc={rc}")
        try:
            yield
        finally:
            n = lib.axon_stop_nrt_profile(str(output_dir).encode())
            print(f"profile: {n} file(s) written to {output_dir}")

    mod = types.ModuleType("antenv.axon_hooks")
    mod.get_axon_ntff_profile_hook = lambda: _hook
    mod.set_axon_ntff_profile_hook = lambda h: None
    _sys.modules["antenv.axon_hooks"] = mod


def _gather_core(r):
    """Device out (NG, 84, 4, NWIN*C) bf16 -> orig (BLOC, F, C, L) f32."""
    starts = np.asarray(WIN_STARTS)
    p_flat = (starts[:, None] + np.arange(W)[None, :]).ravel()
    rr = (
        np.asarray(r, dtype=np.float32)
        .reshape(NG, F, W, 4, NWIN, C)
        .transpose(0, 3, 1, 2, 4, 5)
        .reshape(BLOC, F, W, NWIN, C)
    )                                                         # (b, f, l', w, c)
    o = np.empty((BLOC, F, C, L), dtype=np.float32)
    rt = rr.transpose(0, 1, 4, 3, 2).reshape(BLOC, F, C, NWIN * W)
    o[:, :, :, p_flat] = rt
    return o


def kernel(x, filter_params_batch):
    from concourse.bass_utils import run_bass_kernel_spmd

    x = np.asarray(x, dtype=np.float32)
    fp = np.asarray(filter_params_batch, dtype=np.float32)
    taps = _make_taps_np(fp)                                  # (B, 3, K)
    xr = x.reshape(B, C, L)

    nc = _get_program()
    in_maps = []
    for cid in range(NCORES):
        sl = slice(cid * BLOC, (cid + 1) * BLOC)
        in_maps.append(_prep_core_inputs(xr[sl], taps[sl]))

    import os

    trace = bool(int(os.environ.get("KERNEL_TRACE", "0")))
    if trace:
        _install_ntff_hook()
    res = run_bass_kernel_spmd(
        nc, in_maps, core_ids=list(range(NCORES)), trace=trace
    )
    kernel.last_results = res

    outs = [_gather_core(res.results[cid]["out"]) for cid in range(NCORES)]
    orig = np.concatenate(outs, axis=0)                       # (B, F, C, L)

    # grouped-conv channel routing: out[b, f, c] = orig[b, f, (32 f + c)//3]
    m = np.arange(C * F)
    ch = (m // F).reshape(F, C)                               # (3, 32)
    out = orig[:, np.arange(F)[:, None], ch, :]               # (B, F, C, L)
    return np.ascontiguousarray(out.astype(np.float32))


kernel.last_results = None

